# revision 16
# baseline (speedup 1.0000x reference)
"""BiLinearAttention TRN2 Bass kernel.

Math (per batch element n, data-parallel over 8 NeuronCores):
    q_proj = query @ W.T + b          # [L, D]
    score  = q_proj @ key.T           # [L, S]
    P      = softmax(score, axis=-1)
    out    = P @ value                # [L, D]

Shapes: query/key/value [2048, 1024] f32 per core, W [1024, 1024], b [1024].

Design notes (all HW-verified on TRN2):
  - fp32 matmuls cost 4 cycles/row on the PE; 16-bit matmuls cost 1.
    All three matmuls run as SINGLE-PASS fp16 (operands rounded to fp16,
    fp32 PSUM accumulation). Error budget (numpy-emulated on the real
    distributions): logit noise ~0.02 absolute against top-2 gaps of
    ~15 mean / 0.13 at p1 -> end-to-end L2 rel err 2.6e-3, well under
    the 2e-2 gate. The old hi/lo 3-pass splits bought 2e-4 accuracy at
    3x the PE cost of the proj and score matmuls.
  - No PE transposes: operands are rounded to fp16 in natural layout
    (cheap free-dim DVE copy) and moved to contraction-major layout
    with the 2-byte X-bar DMA transpose, batched as one
    [128, F] -> [128, F/128, 128] descriptor set per tile row.
  - Engine-queue discipline: a sequencer blocks on its current
    instruction's semaphore waits, so dependent DMAs interleaved on one
    queue serialize the whole prep pipeline. Prep loads issue in groups
    of 4 ahead of the group's X-bars; X-bar transposes all stay on SP
    (concurrent X-bar streams from two HWDGE queues corrupt data --
    HW-verified); stores ride GPSIMD/SWDGE.
  - Softmax over s in [l, s] layout: free-dim reduce_max on DVE, exp on
    ACT reading score PSUM directly, with accum_out producing the
    denominator. P is emitted as fp16 scaled by 2^10 (folded into the
    exp bias; the normalizer absorbs it) to keep the tail of the
    near-one-hot distribution out of fp16 denormals.
  - P tiles X-bar-transposed, P.T @ value in fp16, then
    out = psum * (1/sum) via per-partition tensor_scalar on DVE.
"""

import numpy as np
from contextlib import ExitStack

import concourse.bass as bass
import concourse.tile as tile
from concourse import mybir, bacc, bass_utils

F32 = mybir.dt.float32
F16 = mybir.dt.float16
AF = mybir.ActivationFunctionType
AX = mybir.AxisListType

N, L, S, D = 8, 2048, 2048, 1024
N_CORES = 8
LT = L // 128       # 16 l tiles
ST = S // 128       # 16 s tiles
KC = D // 128       # 8 contraction chunks (both q and k dims)
SB = S // 512       # 4 score blocks per l tile
LB = L // 512       # 4 l blocks in projection
DB = D // 512       # 2 d blocks in PV

PSCALE = float(np.log(1024.0))


def _emit(ctx: ExitStack, tc: tile.TileContext,
          query, key, value, W, b, out, loop_T=0, only=None):
    nc = tc.nc
    _emit.uid = getattr(_emit, "uid", 0)

    base = ctx.enter_context(tc.tile_pool(name="base", bufs=1))
    b_sb = base.tile([128, KC], F32)
    nc.gpsimd.dma_start(b_sb, b.rearrange("(t p) -> p t", p=128))

    # q_projT fp16, [k_in_chunk, k_chunk, l_quarter] -- persistent
    p_qp = ctx.enter_context(tc.tile_pool(name="qp", bufs=1))
    qpT = [p_qp.tile([128, KC, 512], F16, name=f"qpT{i}") for i in range(LB)]

    # keyT quarters + value fp16 -- persistent through phase C
    p_kv = ctx.enter_context(tc.tile_pool(name="kv", bufs=1))
    kT = [p_kv.tile([128, KC, 512], F16, name=f"kT{i}") for i in range(4)]
    v_sb = [p_kv.tile([128, 4, D], F16, name=f"vsb{i}") for i in range(4)]

    # ------- prep ---------------------------------------------------------
    # Measured ring rates (this box): SWDGE ~180-210 GB/s with 2-8MB DMAs,
    # scalar/sync HWDGE ~80-100 GB/s, X-bar ~90 GB/s, casting loads
    # ~114 GB/s read-side; rings aggregate to only ~200-230 GB/s. So: the
    # proj/score-critical tensors (W, query, key) ride SWDGE as big plain
    # f32 DMAs in arrival order, value rides scalar in parallel (needed
    # last), DVE rounds everything to fp16, and the sync ring carries ONLY
    # X-bar transposes (concurrent X-bars from two HWDGE queues corrupt
    # data -- HW-verified).
    with tc.tile_pool(name="wt", bufs=1) as p_wt, \
         tc.tile_pool(name="vstage", bufs=2) as p_vs, \
         tc.tile_pool(name="qtb", bufs=2) as p_qtb, \
         tc.tile_pool(name="ps_mm", bufs=4, space="PSUM") as ps_mm:

        WT = [p_wt.tile([128, KC, 128], F16, name=f"WT{kt}") for kt in range(KC)]

        # scalar ring: value f32 2MB chunks, from t=0
        vnat = []
        for vq in range(4):
            vn = p_vs.tile([128, 4, D], F32, tag="vn", name=f"vn{vq}")
            nc.scalar.dma_start(
                vn, value.rearrange("(t p) d -> p t d", p=128)[:, vq * 4:(vq + 1) * 4, :])
            vnat.append(vn)

        def round_v(vq):
            # DVE round of value chunk vq, emitted at a point where its
            # scalar-ring load has landed (keeps vstage at bufs=2)
            for t in range(4):
                nc.vector.tensor_copy(v_sb[vq][:, t, :], vnat[vq][:, t, :])

        # SWDGE: W as two 2MB DMAs (one staging buffer, WAR-rotated)
        with tc.tile_pool(name="wstage", bufs=1) as p_ws:
            w16 = p_ws.tile([128, KC, D], F16)
            for g in range(2):
                wnat = p_ws.tile([128, 4, D], F32, tag="wn", name=f"wn{g}")
                nc.gpsimd.dma_start(
                    wnat, W.rearrange("(t p) d -> p t d", p=128)[:, g * 4:(g + 1) * 4, :])
                for i in range(4):
                    kt = g * 4 + i
                    nc.vector.tensor_copy(w16[:, kt, :], wnat[:, i, :])
                    nc.sync.dma_start(WT[kt], w16[:, kt, :], transpose=True)
        round_v(0)

        # SWDGE: query 2MB chunks (one per l block); round; xbar; proj
        with tc.tile_pool(name="qstage", bufs=2) as p_qs, \
             tc.tile_pool(name="qstage16", bufs=1) as p_qs16:
            for lb in range(LB):
                qn = p_qs.tile([128, 4, D], F32, tag="qn")
                nc.gpsimd.dma_start(
                    qn, query.rearrange("(t p) d -> p t d", p=128)[:, lb * 4:(lb + 1) * 4, :])
                qT = p_qtb.tile([128, KC, 512], F16, tag="qT")
                q16 = p_qs16.tile([128, 4, D], F16, tag="q16")
                for i in range(4):
                    nc.vector.tensor_copy(q16[:, i, :], qn[:, i, :])
                    nc.sync.dma_start(qT[:, :, i * 128:(i + 1) * 128],
                                      q16[:, i, :], transpose=True)
                if lb in (1, 3):
                    round_v(1 if lb == 1 else 2)

                # q_projT[k, l_blk] = sum_q W[k, q] * queryT[q, l_blk]
                for kt in range(KC):
                    mm = ps_mm.tile([128, 512], F32, tag="mm")
                    for qc in range(KC):
                        nc.tensor.matmul(mm, WT[kt][:, qc, :], qT[:, qc, :],
                                         start=(qc == 0), stop=(qc == KC - 1))
                    # bias add + fp16 round straight out of PSUM on ACT
                    nc.scalar.activation(qpT[lb][:, kt, :], mm, AF.Identity,
                                         bias=b_sb[:, kt:kt + 1], scale=1.0)

        # SWDGE: key 2MB chunks behind query; round; xbar into kT quarters
        with tc.tile_pool(name="kstage", bufs=2) as p_ks, \
             tc.tile_pool(name="kstage16", bufs=1) as p_ks16:
            for q4 in range(4):
                kn = p_ks.tile([128, 4, D], F32, tag="kn")
                nc.gpsimd.dma_start(
                    kn, key.rearrange("(t p) d -> p t d", p=128)[:, q4 * 4:(q4 + 1) * 4, :])
                k16 = p_ks16.tile([128, 4, D], F16, tag="k16")
                for r4 in range(4):
                    nc.vector.tensor_copy(k16[:, r4, :], kn[:, r4, :])
                    nc.sync.dma_start(kT[q4][:, :, r4 * 128:(r4 + 1) * 128],
                                      k16[:, r4, :], transpose=True)
                if q4 == 1:
                    round_v(3)

    if only == "prep":
        return

    # ------- phase C: attention over l tiles -------
    ps_score = ctx.enter_context(tc.tile_pool(name="ps_s", bufs=5, space="PSUM"))
    ps_out = ctx.enter_context(tc.tile_pool(name="ps_o", bufs=2, space="PSUM"))
    p_p = ctx.enter_context(tc.tile_pool(name="p_p", bufs=2))
    p_pt = ctx.enter_context(tc.tile_pool(name="p_pt", bufs=2))
    p_stat = ctx.enter_context(tc.tile_pool(name="p_stat", bufs=3))
    p_out = ctx.enter_context(tc.tile_pool(name="p_out", bufs=2))

    def emit_score_softmax(lt):
        """Score matmuls + softmax for l tile lt; returns (PT, 1/sum)."""
        score_ps = []
        mx4 = p_stat.tile([128, SB], F32, tag="mx4")
        lb, li = divmod(lt, 4)
        lsl = slice(li * 128, (li + 1) * 128)
        for sb in range(SB):
            mm = ps_score.tile([128, 512], F32, tag="sc")
            for kc in range(KC):
                nc.tensor.matmul(mm, qpT[lb][:, kc, lsl], kT[sb][:, kc, :],
                                 start=(kc == 0), stop=(kc == KC - 1))
            nc.vector.reduce_max(mx4[:, sb:sb + 1], mm, axis=AX.X)
            score_ps.append(mm)

        nm = p_stat.tile([128, 1], F32, tag="nm")
        # nm = -(max) + ln(2^10): P scaled by 1024 (normalizer absorbs it)
        nc.vector.reduce_max(nm, mx4, axis=AX.X, negate=True)
        nc.vector.tensor_scalar_add(nm, nm, PSCALE)
        p_sb = p_p.tile([128, S], F16, tag="p")
        ssum4 = p_stat.tile([128, SB], F32, tag="ssum4")
        for sb in range(SB):
            nc.scalar.activation(p_sb[:, sb * 512:(sb + 1) * 512], score_ps[sb],
                                 AF.Exp, bias=nm, scale=1.0,
                                 accum_out=ssum4[:, sb:sb + 1])
        ssum = p_stat.tile([128, 1], F32, tag="ssum")
        nc.vector.reduce_sum(ssum, ssum4, axis=AX.X)
        rinv = p_stat.tile([128, 1], F32, tag="rinv")
        nc.vector.reciprocal(rinv, ssum)
        # PT[s', sc, l'] = P[l', sc*128+s'] -- one batched xbar transpose
        pt = p_pt.tile([128, ST, 128], F16, tag="pt")
        nc.sync.dma_start(pt, p_sb, transpose=True)
        return pt, rinv

    def emit_pv(lt, pt, rinv):
        """P.T-weighted V accumulation, scale, store."""
        out_ps = [ps_out.tile([128, 512], F32, tag="o", name=f"ops{lt}_{i}")
                  for i in range(DB)]
        for sc in range(ST):
            for dc in range(DB):
                nc.tensor.matmul(out_ps[dc], pt[:, sc, :],
                                 v_sb[sc // 4][:, sc % 4, dc * 512:(dc + 1) * 512],
                                 start=(sc == 0), stop=(sc == ST - 1))
        o_sb = p_out.tile([128, D], F32, tag="osb")
        for dc in range(DB):
            nc.vector.tensor_scalar_mul(o_sb[:, dc * 512:(dc + 1) * 512],
                                        out_ps[dc], rinv)
        nc.gpsimd.dma_start(out[lt * 128:(lt + 1) * 128, :], o_sb)

    def phase4():
        pending = None
        for lt in range(LT):
            cur = emit_score_softmax(lt)
            if pending is not None:
                emit_pv(lt - 1, *pending)
            pending = cur
        emit_pv(LT - 1, *pending)

    if loop_T:
        with tc.For_i(0, loop_T, 1):
            phase4()
    else:
        phase4()


# ---------------- v5: steady-state software-pipelined loop path ----------
# For the on-device benchmark loop: each For_i body is [gap | phase C] where
# the gap runs only the projection (+ the last key-half transpose chain) and
# phase C hides ALL next-iteration input staging:
#   - SWDGE casting loads (f32->f16 in-DMA, ~114 GB/s read) stream value,
#     W, query halves and key quarters 0-1 into SBUF arenas N0/N1 while the
#     PE runs score/PV at its roofline;
#   - the sync ring X-bar-transposes W/query/key-0-1 into WT/qT/kT between
#     PT transposes (their former contents are dead: proj consumed qT/WT in
#     the gap, score lt15 released kT);
#   - key half 2 rides the scalar HWDGE ring as f32 in the gap (+DVE round)
#     because SWDGE is saturated; out stores also ride scalar.
# The For_i back-edge is a full all-engine barrier, which makes every
# cross-iteration read of a staged tile safe by construction.

def _v5_state(ctx, tc):
    nc = tc.nc
    p = ctx.enter_context(tc.tile_pool(name="v5persist", bufs=1))
    st = {
        "b_sb": p.tile([128, KC], F32, name="b_sb"),
        "qpT": [p.tile([128, KC, 512], F16, name=f"qpT{i}") for i in range(LB)],
        "kT": [p.tile([128, 4, KC, 128], F16, name=f"kT{i}") for i in range(4)],
        "v_sb": [p.tile([128, 4, D], F16, name=f"vsb{i}") for i in range(4)],
        "WT": p.tile([128, KC, KC, 128], F16, name="WTall"),
        "N0": p.tile([128, KC, D], F16, name="N0"),
        "N1": p.tile([128, KC, D], F16, name="N1"),
        "qT": [p.tile([128, 4, KC, 128], F16, name=f"qTt{i}") for i in range(LB)],
    }
    st["p_p"] = ctx.enter_context(tc.tile_pool(name="p_p", bufs=1))
    st["p_pt"] = ctx.enter_context(tc.tile_pool(name="p_pt", bufs=2))
    st["p_stat"] = ctx.enter_context(tc.tile_pool(name="p_stat", bufs=3))
    st["p_out"] = ctx.enter_context(tc.tile_pool(name="p_out", bufs=1))
    return st


def _v5_xbar_qT(nc, st, lb):
    src = (st["N0"] if lb < 2 else st["N1"])[:, (lb % 2) * 4:(lb % 2) * 4 + 4, :]
    nc.sync.dma_start(st["qT"][lb], src, transpose=True)


def _v5_xbar_kT(nc, st, q):
    src = st["N0"][:, (q % 2) * 4:(q % 2) * 4 + 4, :]
    nc.sync.dma_start(st["kT"][q], src, transpose=True)


def _v5_prologue(tc, st, query, key, W, b):
    nc = tc.nc
    kre = key.rearrange("(t p) d -> p t d", p=128)
    qre = query.rearrange("(t p) d -> p t d", p=128)
    nc.gpsimd.dma_start(st["b_sb"], b.rearrange("(t p) -> p t", p=128))
    nc.gpsimd.dma_start(st["N1"], W.rearrange("(t p) d -> p t d", p=128))
    nc.sync.dma_start(st["WT"], st["N1"], transpose=True)
    nc.gpsimd.dma_start(st["N0"], qre[:, 0:8, :])
    _v5_xbar_qT(nc, st, 0)
    _v5_xbar_qT(nc, st, 1)
    nc.gpsimd.dma_start(st["N1"], qre[:, 8:16, :])
    _v5_xbar_qT(nc, st, 2)
    _v5_xbar_qT(nc, st, 3)
    nc.gpsimd.dma_start(st["N0"][:, 0:4, :], kre[:, 0:4, :])
    _v5_xbar_kT(nc, st, 0)
    nc.gpsimd.dma_start(st["N0"][:, 4:8, :], kre[:, 4:8, :])
    _v5_xbar_kT(nc, st, 1)


def _v5_body(ctx, tc, st, query, key, value, W, b, out):
    nc = tc.nc
    b_sb, qpT, kT, v_sb, WT = (st[k] for k in ("b_sb", "qpT", "kT", "v_sb", "WT"))
    N0, N1 = st["N0"], st["N1"]
    kre = key.rearrange("(t p) d -> p t d", p=128)
    qre = query.rearrange("(t p) d -> p t d", p=128)
    vre = value.rearrange("(t p) d -> p t d", p=128)
    wre = W.rearrange("(t p) d -> p t d", p=128)

    # ---------------- gap ----------------
    # SWDGE: value casts (needed by the first PV, ~14us into phase C)
    for vq in range(4):
        nc.gpsimd.dma_start(v_sb[vq], vre[:, vq * 4:(vq + 1) * 4, :])
    # scalar ring: key half 2 (quarters 2-3) f32 chunks; DVE rounds into
    # N0 (its q01 content was consumed by last iteration's qT xbars);
    # sync: kT[2]/kT[3] xbars (kT dead since last iteration's score lt15)
    with tc.tile_pool(name="k2stage", bufs=2) as p_k2:
        for q in (2, 3):
            for c in range(4):
                t = 8 + (q - 2) * 4 + c
                k2 = p_k2.tile([128, D], F32, tag="k2")
                nc.scalar.dma_start(k2, kre[:, t, :])
                nc.vector.tensor_copy(N0[:, (q % 2) * 4 + c, :], k2)
            _v5_xbar_kT(nc, st, q)

    # PE: projection (qT/WT staged during the previous phase C)
    with tc.tile_pool(name="ps_mm", bufs=4, space="PSUM") as ps_mm:
        for lb in range(LB):
            for kt in range(KC):
                mm = ps_mm.tile([128, 512], F32, tag="mm")
                for qc in range(KC):
                    nc.tensor.matmul(mm, WT[:, kt, qc, :], st["qT"][lb][:, :, qc, :],
                                     start=(qc == 0), stop=(qc == KC - 1))
                nc.scalar.activation(qpT[lb][:, kt, :], mm, AF.Identity,
                                     bias=b_sb[:, kt:kt + 1], scale=1.0)

    # ---------------- phase C (with next-iteration staging) ----------------
    ps_score = ctx.enter_context(tc.tile_pool(name="ps_s", bufs=5, space="PSUM"))
    ps_out = ctx.enter_context(tc.tile_pool(name="ps_o", bufs=2, space="PSUM"))
    p_p, p_pt, p_stat, p_out = (st[k] for k in ("p_p", "p_pt", "p_stat", "p_out"))

    def prefetch(lt):
        # SWDGE cast order: (v from gap), w16, q01, q23, kq0, kq1; sync
        # xbars slotted so each lands after its cast completes
        if lt == 0:
            nc.gpsimd.dma_start(N1, wre)                      # w16
        elif lt == 4:
            nc.sync.dma_start(WT, N1, transpose=True)
            nc.gpsimd.dma_start(N0, qre[:, 0:8, :])           # q01
        elif lt == 6:
            nc.gpsimd.dma_start(N1, qre[:, 8:16, :])          # q23
        elif lt == 8:
            _v5_xbar_qT(nc, st, 0)
        elif lt == 9:
            _v5_xbar_qT(nc, st, 1)
        elif lt == 11:
            _v5_xbar_qT(nc, st, 2)
        elif lt == 12:
            _v5_xbar_qT(nc, st, 3)
            nc.gpsimd.dma_start(N0[:, 0:4, :], kre[:, 0:4, :])  # kq0
        elif lt == 13:
            nc.gpsimd.dma_start(N0[:, 4:8, :], kre[:, 4:8, :])  # kq1

    def emit_score_softmax(lt):
        score_ps = []
        mx4 = p_stat.tile([128, SB], F32, tag="mx4")
        lb, li = divmod(lt, 4)
        lsl = slice(li * 128, (li + 1) * 128)
        for sb in range(SB):
            mm = ps_score.tile([128, 512], F32, tag="sc")
            for kc in range(KC):
                nc.tensor.matmul(mm, qpT[lb][:, kc, lsl], kT[sb][:, :, kc, :],
                                 start=(kc == 0), stop=(kc == KC - 1))
            nc.vector.reduce_max(mx4[:, sb:sb + 1], mm, axis=AX.X)
            score_ps.append(mm)

        nm = p_stat.tile([128, 1], F32, tag="nm")
        nc.vector.reduce_max(nm, mx4, axis=AX.X, negate=True)
        nc.vector.tensor_scalar_add(nm, nm, PSCALE)
        p_sb = p_p.tile([128, S], F16, tag="p")
        ssum4 = p_stat.tile([128, SB], F32, tag="ssum4")
        for sb in range(SB):
            nc.scalar.activation(p_sb[:, sb * 512:(sb + 1) * 512], score_ps[sb],
                                 AF.Exp, bias=nm, scale=1.0,
                                 accum_out=ssum4[:, sb:sb + 1])
        ssum = p_stat.tile([128, 1], F32, tag="ssum")
        nc.vector.reduce_sum(ssum, ssum4, axis=AX.X)
        rinv = p_stat.tile([128, 1], F32, tag="rinv")
        nc.vector.reciprocal(rinv, ssum)
        pt = p_pt.tile([128, ST, 128], F16, tag="pt")
        nc.sync.dma_start(pt, p_sb, transpose=True)
        return pt, rinv

    def emit_pv(lt, pt, rinv):
        out_ps = [ps_out.tile([128, 512], F32, tag="o", name=f"ops{lt}_{i}")
                  for i in range(DB)]
        for sc in range(ST):
            for dc in range(DB):
                nc.tensor.matmul(out_ps[dc], pt[:, sc, :],
                                 v_sb[sc // 4][:, sc % 4, dc * 512:(dc + 1) * 512],
                                 start=(sc == 0), stop=(sc == ST - 1))
        o_sb = p_out.tile([128, D], F32, tag="osb")
        for dc in range(DB):
            nc.vector.tensor_scalar_mul(o_sb[:, dc * 512:(dc + 1) * 512],
                                        out_ps[dc], rinv)
        nc.scalar.dma_start(out[lt * 128:(lt + 1) * 128, :], o_sb)

    pending = None
    for lt in range(LT):
        cur = emit_score_softmax(lt)
        prefetch(lt)
        if pending is not None:
            emit_pv(lt - 1, *pending)
        pending = cur
    # key quarters 0-1 transposes: kT released by score lt15 just above
    _v5_xbar_kT(nc, st, 0)
    _v5_xbar_kT(nc, st, 1)
    emit_pv(LT - 1, *pending)


_CACHE = {}


def _build(reps=1, loop_T=0, loop_all=0, only=None):
    key_ = (reps, loop_T, loop_all, only)
    if key_ in _CACHE:
        return _CACHE[key_]
    nc = bacc.Bacc("TRN2", target_bir_lowering=False, debug=False,
                   num_devices=N_CORES)
    query = nc.dram_tensor("query", [L, D], F32, kind="ExternalInput").ap()
    key = nc.dram_tensor("key", [S, D], F32, kind="ExternalInput").ap()
    value = nc.dram_tensor("value", [S, D], F32, kind="ExternalInput").ap()
    W = nc.dram_tensor("W", [D, D], F32, kind="ExternalInput").ap()
    b = nc.dram_tensor("b", [D], F32, kind="ExternalInput").ap()
    out = nc.dram_tensor("out", [L, D], F32, kind="ExternalOutput").ap()
    tag = None
    loop_T = loop_T or loop_all
    if reps > 1 or loop_T:
        # distinct I/O signature per variant so the neuron compile cache
        # (keyed on HLO structure, not backend_config) can't collide
        tag = nc.dram_tensor("tag", [8, reps * 100 + max(loop_T, 1)], F32,
                             kind="ExternalOutput").ap()
    with tile.TileContext(nc) as tc:
        if loop_all and only is None:
            # steady-state software-pipelined loop (see v5 notes above)
            with ExitStack() as ctx:
                st = _v5_state(ctx, tc)
                _v5_prologue(tc, st, query, key, W, b)
                with tc.For_i(0, loop_all, 1):
                    with ExitStack() as bctx:
                        _v5_body(bctx, tc, st, query, key, value, W, b, out)
        elif loop_all:
            with tc.For_i(0, loop_all, 1):
                with ExitStack() as ctx:
                    _emit(ctx, tc, query, key, value, W, b, out, only=only)
        else:
            for _ in range(reps):
                with ExitStack() as ctx:
                    _emit(ctx, tc, query, key, value, W, b, out, loop_T=loop_T,
                          only=only)
        if tag is not None:
            with tc.tile_pool(name="tagp", bufs=1) as tp:
                t = tp.tile([8, reps * 100 + max(loop_T, 1)], F32)
                nc.vector.memset(t, 1.0)
                nc.sync.dma_start(tag, t)
    nc.compile()
    _CACHE[key_] = nc
    return nc


def kernel(key, query, value, W, b):
    key = np.ascontiguousarray(np.asarray(key), dtype=np.float32)
    query = np.ascontiguousarray(np.asarray(query), dtype=np.float32)
    value = np.ascontiguousarray(np.asarray(value), dtype=np.float32)
    W = np.ascontiguousarray(np.asarray(W), dtype=np.float32)
    b = np.ascontiguousarray(np.asarray(b), dtype=np.float32)
    nc = _build()
    in_maps = [
        {"query": query[i], "key": key[i], "value": value[i], "W": W, "b": b}
        for i in range(N_CORES)
    ]
    res = bass_utils.run_bass_kernel_spmd(nc, in_maps, core_ids=list(range(N_CORES)))
    return np.stack([res.results[i]["out"] for i in range(N_CORES)], axis=0)


# revision 18
# speedup vs baseline: 1.0607x; 1.0607x over previous
"""BiLinearAttention TRN2 Bass kernel.

Math (per batch element n, data-parallel over 8 NeuronCores):
    q_proj = query @ W.T + b          # [L, D]
    score  = q_proj @ key.T           # [L, S]
    P      = softmax(score, axis=-1)
    out    = P @ value                # [L, D]

Shapes: query/key/value [2048, 1024] f32 per core, W [1024, 1024], b [1024].

Design notes (all HW-verified on TRN2):
  - fp32 matmuls cost 4 cycles/row on the PE; 16-bit matmuls cost 1.
    All three matmuls run as SINGLE-PASS fp16 (operands rounded to fp16,
    fp32 PSUM accumulation). Error budget (numpy-emulated on the real
    distributions): logit noise ~0.02 absolute against top-2 gaps of
    ~15 mean / 0.13 at p1 -> end-to-end L2 rel err 2.6e-3, well under
    the 2e-2 gate. The old hi/lo 3-pass splits bought 2e-4 accuracy at
    3x the PE cost of the proj and score matmuls.
  - No PE transposes: operands are rounded to fp16 in natural layout
    (cheap free-dim DVE copy) and moved to contraction-major layout
    with the 2-byte X-bar DMA transpose, batched as one
    [128, F] -> [128, F/128, 128] descriptor set per tile row.
  - Engine-queue discipline: a sequencer blocks on its current
    instruction's semaphore waits, so dependent DMAs interleaved on one
    queue serialize the whole prep pipeline. Prep loads issue in groups
    of 4 ahead of the group's X-bars; X-bar transposes all stay on SP
    (concurrent X-bar streams from two HWDGE queues corrupt data --
    HW-verified); stores ride GPSIMD/SWDGE.
  - Softmax over s in [l, s] layout: free-dim reduce_max on DVE, exp on
    ACT reading score PSUM directly, with accum_out producing the
    denominator. P is emitted as fp16 scaled by 2^10 (folded into the
    exp bias; the normalizer absorbs it) to keep the tail of the
    near-one-hot distribution out of fp16 denormals.
  - P tiles X-bar-transposed, P.T @ value in fp16, then
    out = psum * (1/sum) via per-partition tensor_scalar on DVE.
"""

import numpy as np
from contextlib import ExitStack

import concourse.bass as bass
import concourse.tile as tile
from concourse import mybir, bacc, bass_utils

F32 = mybir.dt.float32
F16 = mybir.dt.float16
AF = mybir.ActivationFunctionType
AX = mybir.AxisListType

N, L, S, D = 8, 2048, 2048, 1024
N_CORES = 8
LT = L // 128       # 16 l tiles
ST = S // 128       # 16 s tiles
KC = D // 128       # 8 contraction chunks (both q and k dims)
SB = S // 512       # 4 score blocks per l tile
LB = L // 512       # 4 l blocks in projection
DB = D // 512       # 2 d blocks in PV

PSCALE = float(np.log(1024.0))


def _emit(ctx: ExitStack, tc: tile.TileContext,
          query, key, value, W, b, out, loop_T=0, only=None):
    nc = tc.nc
    _emit.uid = getattr(_emit, "uid", 0)

    base = ctx.enter_context(tc.tile_pool(name="base", bufs=1))
    b_sb = base.tile([128, KC], F32)
    nc.gpsimd.dma_start(b_sb, b.rearrange("(t p) -> p t", p=128))

    # q_projT fp16, [k_in_chunk, k_chunk, l_quarter] -- persistent
    p_qp = ctx.enter_context(tc.tile_pool(name="qp", bufs=1))
    qpT = [p_qp.tile([128, KC, 512], F16, name=f"qpT{i}") for i in range(LB)]

    # keyT quarters + value fp16 -- persistent through phase C
    p_kv = ctx.enter_context(tc.tile_pool(name="kv", bufs=1))
    kT = [p_kv.tile([128, KC, 512], F16, name=f"kT{i}") for i in range(4)]
    v_sb = [p_kv.tile([128, 4, D], F16, name=f"vsb{i}") for i in range(4)]

    # ------- prep ---------------------------------------------------------
    # Measured ring rates (this box): SWDGE ~180-210 GB/s with 2-8MB DMAs,
    # scalar/sync HWDGE ~80-100 GB/s, X-bar ~90 GB/s, casting loads
    # ~114 GB/s read-side; rings aggregate to only ~200-230 GB/s. So: the
    # proj/score-critical tensors (W, query, key) ride SWDGE as big plain
    # f32 DMAs in arrival order, value rides scalar in parallel (needed
    # last), DVE rounds everything to fp16, and the sync ring carries ONLY
    # X-bar transposes (concurrent X-bars from two HWDGE queues corrupt
    # data -- HW-verified).
    with tc.tile_pool(name="wt", bufs=1) as p_wt, \
         tc.tile_pool(name="vstage", bufs=2) as p_vs, \
         tc.tile_pool(name="qtb", bufs=2) as p_qtb, \
         tc.tile_pool(name="ps_mm", bufs=4, space="PSUM") as ps_mm:

        WT = [p_wt.tile([128, KC, 128], F16, name=f"WT{kt}") for kt in range(KC)]

        # scalar ring: value f32 2MB chunks, from t=0
        vnat = []
        for vq in range(4):
            vn = p_vs.tile([128, 4, D], F32, tag="vn", name=f"vn{vq}")
            nc.scalar.dma_start(
                vn, value.rearrange("(t p) d -> p t d", p=128)[:, vq * 4:(vq + 1) * 4, :])
            vnat.append(vn)

        def round_v(vq):
            # DVE round of value chunk vq, emitted at a point where its
            # scalar-ring load has landed (keeps vstage at bufs=2)
            for t in range(4):
                nc.vector.tensor_copy(v_sb[vq][:, t, :], vnat[vq][:, t, :])

        # SWDGE: W as two 2MB DMAs (one staging buffer, WAR-rotated)
        with tc.tile_pool(name="wstage", bufs=1) as p_ws:
            w16 = p_ws.tile([128, KC, D], F16)
            for g in range(2):
                wnat = p_ws.tile([128, 4, D], F32, tag="wn", name=f"wn{g}")
                nc.gpsimd.dma_start(
                    wnat, W.rearrange("(t p) d -> p t d", p=128)[:, g * 4:(g + 1) * 4, :])
                for i in range(4):
                    kt = g * 4 + i
                    nc.vector.tensor_copy(w16[:, kt, :], wnat[:, i, :])
                    nc.sync.dma_start(WT[kt], w16[:, kt, :], transpose=True)
        round_v(0)

        # SWDGE: query 2MB chunks (one per l block); round; xbar; proj
        with tc.tile_pool(name="qstage", bufs=2) as p_qs, \
             tc.tile_pool(name="qstage16", bufs=1) as p_qs16:
            for lb in range(LB):
                qn = p_qs.tile([128, 4, D], F32, tag="qn")
                nc.gpsimd.dma_start(
                    qn, query.rearrange("(t p) d -> p t d", p=128)[:, lb * 4:(lb + 1) * 4, :])
                qT = p_qtb.tile([128, KC, 512], F16, tag="qT")
                q16 = p_qs16.tile([128, 4, D], F16, tag="q16")
                for i in range(4):
                    nc.vector.tensor_copy(q16[:, i, :], qn[:, i, :])
                    nc.sync.dma_start(qT[:, :, i * 128:(i + 1) * 128],
                                      q16[:, i, :], transpose=True)
                if lb in (1, 3):
                    round_v(1 if lb == 1 else 2)

                # q_projT[k, l_blk] = sum_q W[k, q] * queryT[q, l_blk]
                for kt in range(KC):
                    mm = ps_mm.tile([128, 512], F32, tag="mm")
                    for qc in range(KC):
                        nc.tensor.matmul(mm, WT[kt][:, qc, :], qT[:, qc, :],
                                         start=(qc == 0), stop=(qc == KC - 1))
                    # bias add + fp16 round straight out of PSUM on ACT
                    nc.scalar.activation(qpT[lb][:, kt, :], mm, AF.Identity,
                                         bias=b_sb[:, kt:kt + 1], scale=1.0)

        # SWDGE: key 2MB chunks behind query; round; xbar into kT quarters
        with tc.tile_pool(name="kstage", bufs=2) as p_ks, \
             tc.tile_pool(name="kstage16", bufs=1) as p_ks16:
            for q4 in range(4):
                kn = p_ks.tile([128, 4, D], F32, tag="kn")
                nc.gpsimd.dma_start(
                    kn, key.rearrange("(t p) d -> p t d", p=128)[:, q4 * 4:(q4 + 1) * 4, :])
                k16 = p_ks16.tile([128, 4, D], F16, tag="k16")
                for r4 in range(4):
                    nc.vector.tensor_copy(k16[:, r4, :], kn[:, r4, :])
                    nc.sync.dma_start(kT[q4][:, :, r4 * 128:(r4 + 1) * 128],
                                      k16[:, r4, :], transpose=True)
                if q4 == 1:
                    round_v(3)

    if only == "prep":
        return

    # ------- phase C: attention over l tiles -------
    ps_score = ctx.enter_context(tc.tile_pool(name="ps_s", bufs=5, space="PSUM"))
    ps_out = ctx.enter_context(tc.tile_pool(name="ps_o", bufs=2, space="PSUM"))
    p_p = ctx.enter_context(tc.tile_pool(name="p_p", bufs=2))
    p_pt = ctx.enter_context(tc.tile_pool(name="p_pt", bufs=2))
    p_stat = ctx.enter_context(tc.tile_pool(name="p_stat", bufs=3))
    p_out = ctx.enter_context(tc.tile_pool(name="p_out", bufs=2))

    def emit_score_softmax(lt):
        """Score matmuls + softmax for l tile lt; returns (PT, 1/sum)."""
        score_ps = []
        mx4 = p_stat.tile([128, SB], F32, tag="mx4")
        lb, li = divmod(lt, 4)
        lsl = slice(li * 128, (li + 1) * 128)
        for sb in range(SB):
            mm = ps_score.tile([128, 512], F32, tag="sc")
            for kc in range(KC):
                nc.tensor.matmul(mm, qpT[lb][:, kc, lsl], kT[sb][:, kc, :],
                                 start=(kc == 0), stop=(kc == KC - 1))
            nc.vector.reduce_max(mx4[:, sb:sb + 1], mm, axis=AX.X)
            score_ps.append(mm)

        nm = p_stat.tile([128, 1], F32, tag="nm")
        # nm = -(max) + ln(2^10): P scaled by 1024 (normalizer absorbs it)
        nc.vector.reduce_max(nm, mx4, axis=AX.X, negate=True)
        nc.vector.tensor_scalar_add(nm, nm, PSCALE)
        p_sb = p_p.tile([128, S], F16, tag="p")
        ssum4 = p_stat.tile([128, SB], F32, tag="ssum4")
        for sb in range(SB):
            nc.scalar.activation(p_sb[:, sb * 512:(sb + 1) * 512], score_ps[sb],
                                 AF.Exp, bias=nm, scale=1.0,
                                 accum_out=ssum4[:, sb:sb + 1])
        ssum = p_stat.tile([128, 1], F32, tag="ssum")
        nc.vector.reduce_sum(ssum, ssum4, axis=AX.X)
        rinv = p_stat.tile([128, 1], F32, tag="rinv")
        nc.vector.reciprocal(rinv, ssum)
        # PT[s', sc, l'] = P[l', sc*128+s'] -- one batched xbar transpose
        pt = p_pt.tile([128, ST, 128], F16, tag="pt")
        nc.sync.dma_start(pt, p_sb, transpose=True)
        return pt, rinv

    def emit_pv(lt, pt, rinv):
        """P.T-weighted V accumulation, scale, store."""
        out_ps = [ps_out.tile([128, 512], F32, tag="o", name=f"ops{lt}_{i}")
                  for i in range(DB)]
        for sc in range(ST):
            for dc in range(DB):
                nc.tensor.matmul(out_ps[dc], pt[:, sc, :],
                                 v_sb[sc // 4][:, sc % 4, dc * 512:(dc + 1) * 512],
                                 start=(sc == 0), stop=(sc == ST - 1))
        o_sb = p_out.tile([128, D], F32, tag="osb")
        for dc in range(DB):
            nc.vector.tensor_scalar_mul(o_sb[:, dc * 512:(dc + 1) * 512],
                                        out_ps[dc], rinv)
        nc.gpsimd.dma_start(out[lt * 128:(lt + 1) * 128, :], o_sb)

    def phase4():
        pending = None
        for lt in range(LT):
            cur = emit_score_softmax(lt)
            if pending is not None:
                emit_pv(lt - 1, *pending)
            pending = cur
        emit_pv(LT - 1, *pending)

    if loop_T:
        with tc.For_i(0, loop_T, 1):
            phase4()
    else:
        phase4()


# ---------------- v5: steady-state software-pipelined loop path ----------
# For the on-device benchmark loop: each For_i body is [gap | phase C] where
# the gap runs only the projection (+ the last key-half transpose chain) and
# phase C hides ALL next-iteration input staging:
#   - SWDGE casting loads (f32->f16 in-DMA, ~114 GB/s read) stream value,
#     W, query halves and key quarters 0-1 into SBUF arenas N0/N1 while the
#     PE runs score/PV at its roofline;
#   - the sync ring X-bar-transposes W/query/key-0-1 into WT/qT/kT between
#     PT transposes (their former contents are dead: proj consumed qT/WT in
#     the gap, score lt15 released kT);
#   - key half 2 rides the scalar HWDGE ring as f32 in the gap (+DVE round)
#     because SWDGE is saturated; out stores also ride scalar.
# The For_i back-edge is a full all-engine barrier, which makes every
# cross-iteration read of a staged tile safe by construction.

def _v5_state(ctx, tc):
    nc = tc.nc
    p = ctx.enter_context(tc.tile_pool(name="v5persist", bufs=1))
    st = {
        "b_sb": p.tile([128, KC], F32, name="b_sb"),
        "qpT": [p.tile([128, KC, 512], F16, name=f"qpT{i}") for i in range(LB)],
        "kT": [p.tile([128, KC, 512], F16, name=f"kT{i}") for i in range(4)],
        "v_sb": [p.tile([128, 4, D], F16, name=f"vsb{i}") for i in range(4)],
        "WT": p.tile([128, KC, KC, 128], F16, name="WTall"),
        "N0": p.tile([128, KC, D], F16, name="N0"),
        "N1": p.tile([128, KC, D], F16, name="N1"),
        "qT": [p.tile([128, KC, 512], F16, name=f"qTt{i}") for i in range(LB)],
    }
    st["ident"] = p.tile([128, 128], F16, name="ident")
    st["p_p"] = ctx.enter_context(tc.tile_pool(name="p_p", bufs=1))
    st["p_pt"] = ctx.enter_context(tc.tile_pool(name="p_pt", bufs=2))
    st["p_stat"] = ctx.enter_context(tc.tile_pool(name="p_stat", bufs=3))
    st["p_out"] = ctx.enter_context(tc.tile_pool(name="p_out", bufs=1))
    return st


def _v5_xbar_qT(nc, st, lb):
    N = st["N0"] if lb < 2 else st["N1"]
    for i in range(4):
        nc.sync.dma_start(st["qT"][lb][:, :, i * 128:(i + 1) * 128],
                          N[:, (lb % 2) * 4 + i, :], transpose=True)


def _v5_pet_kT(tc, st, q, ps_t):
    # PE transpose: kT[q][:, kc, t*128+f] = N[f_row=t of quarter q][kc*128+...]
    nc = tc.nc
    N = st["N1"] if q < 2 else st["N0"]
    for kc in range(KC):
        pst = ps_t.tile([128, 512], F16, tag="pst")
        for t in range(4):
            nc.tensor.transpose(pst[:, t * 128:(t + 1) * 128],
                                N[:, (q % 2) * 4 + t, kc * 128:(kc + 1) * 128],
                                st["ident"])
        nc.vector.tensor_copy(st["kT"][q][:, kc, :], pst)


def _v5_prologue(tc, st, query, key, W, b):
    from concourse import masks
    nc = tc.nc
    kre = key.rearrange("(t p) d -> p t d", p=128)
    qre = query.rearrange("(t p) d -> p t d", p=128)
    nc.gpsimd.dma_start(st["b_sb"], b.rearrange("(t p) -> p t", p=128))
    masks.make_identity(nc, st["ident"][:])
    nc.gpsimd.dma_start(st["N1"], W.rearrange("(t p) d -> p t d", p=128))
    nc.sync.dma_start(st["WT"], st["N1"], transpose=True)
    nc.gpsimd.dma_start(st["N0"], qre[:, 0:8, :])
    _v5_xbar_qT(nc, st, 0)
    _v5_xbar_qT(nc, st, 1)
    nc.gpsimd.dma_start(st["N1"], qre[:, 8:16, :])
    _v5_xbar_qT(nc, st, 2)
    _v5_xbar_qT(nc, st, 3)
    # key quarters 0-1 natural fp16 into N1 (consumed by body gap's PE-T)
    nc.gpsimd.dma_start(st["N1"], kre[:, 0:8, :])


def _v5_body(ctx, tc, st, query, key, value, W, b, out, ablate=None):
    nc = tc.nc
    b_sb, qpT, kT, v_sb, WT = (st[k] for k in ("b_sb", "qpT", "kT", "v_sb", "WT"))
    N0, N1 = st["N0"], st["N1"]
    kre = key.rearrange("(t p) d -> p t d", p=128)
    qre = query.rearrange("(t p) d -> p t d", p=128)
    vre = value.rearrange("(t p) d -> p t d", p=128)
    wre = W.rearrange("(t p) d -> p t d", p=128)

    # ---------------- gap ----------------
    # SWDGE: value casts (needed by the first PV, ~14us into phase C)
    if ablate != "v5c":
        for vq in range(4):
            nc.gpsimd.dma_start(v_sb[vq], vre[:, vq * 4:(vq + 1) * 4, :])
    # scalar ring: key half 2 (quarters 2-3) f32 chunks; DVE rounds into
    # N0 (its q01 content was consumed by last iteration's qT xbars);
    # sync: kT[2]/kT[3] xbars (kT dead since last iteration's score lt15)
    # scalar ring f32 loads + DVE rounds for key quarters 2-3 (into N0)
    with tc.tile_pool(name="k2stage", bufs=2) as p_k2:
        for q in () if ablate == "v5c" else (2, 3):
            for c in range(4):
                t = 8 + (q - 2) * 4 + c
                k2 = p_k2.tile([128, D], F32, tag="k2")
                nc.scalar.dma_start(k2, kre[:, t, :])
                nc.vector.tensor_copy(N0[:, (q % 2) * 4 + c, :], k2)

    # PE: kT transposes (quarters 0-1 from N1, staged last iteration) +
    # projection, then kT quarters 2-3 as their rounds land
    with tc.tile_pool(name="ps_mm", bufs=4, space="PSUM") as ps_mm, \
         tc.tile_pool(name="ps_t", bufs=2, space="PSUM") as ps_t:
        if ablate != "v5c":
            _v5_pet_kT(tc, st, 0, ps_t)
            _v5_pet_kT(tc, st, 1, ps_t)
        for lb in range(LB):
            for kt in range(KC):
                mm = ps_mm.tile([128, 512], F32, tag="mm")
                for qc in range(KC):
                    nc.tensor.matmul(mm, WT[:, kt, qc, :], st["qT"][lb][:, qc, :],
                                     start=(qc == 0), stop=(qc == KC - 1))
                nc.scalar.activation(qpT[lb][:, kt, :], mm, AF.Identity,
                                     bias=b_sb[:, kt:kt + 1], scale=1.0)
            if ablate != "v5c" and lb in (1, 2):
                _v5_pet_kT(tc, st, 2 + (lb - 1), ps_t)

    # ---------------- phase C (with next-iteration staging) ----------------
    ps_score = ctx.enter_context(tc.tile_pool(name="ps_s", bufs=5, space="PSUM"))
    ps_out = ctx.enter_context(tc.tile_pool(name="ps_o", bufs=2, space="PSUM"))
    p_p, p_pt, p_stat, p_out = (st[k] for k in ("p_p", "p_pt", "p_stat", "p_out"))

    def prefetch(lt):
        # SWDGE cast order: (v from gap), w16, q01, q23, kq0, kq1; sync
        # xbars slotted so each lands after its cast completes
        if lt == 0:
            nc.gpsimd.dma_start(N1, wre)                      # w16
        elif lt == 4:
            nc.sync.dma_start(WT, N1, transpose=True)
            nc.gpsimd.dma_start(N0, qre[:, 0:8, :])           # q01
        elif lt == 6:
            nc.gpsimd.dma_start(N1, qre[:, 8:16, :])          # q23
        elif lt == 8:
            _v5_xbar_qT(nc, st, 0)
        elif lt == 9:
            _v5_xbar_qT(nc, st, 1)
        elif lt == 11:
            _v5_xbar_qT(nc, st, 2)
        elif lt == 12:
            _v5_xbar_qT(nc, st, 3)
        elif lt == 13:
            nc.gpsimd.dma_start(N1, kre[:, 0:8, :])  # key quarters 0-1

    def emit_score_softmax(lt):
        score_ps = []
        mx4 = p_stat.tile([128, SB], F32, tag="mx4")
        lb, li = divmod(lt, 4)
        lsl = slice(li * 128, (li + 1) * 128)
        for sb in range(SB):
            mm = ps_score.tile([128, 512], F32, tag="sc")
            for kc in range(KC):
                nc.tensor.matmul(mm, qpT[lb][:, kc, lsl], kT[sb][:, kc, :],
                                 start=(kc == 0), stop=(kc == KC - 1))
            nc.vector.reduce_max(mx4[:, sb:sb + 1], mm, axis=AX.X)
            score_ps.append(mm)

        nm = p_stat.tile([128, 1], F32, tag="nm")
        nc.vector.reduce_max(nm, mx4, axis=AX.X, negate=True)
        nc.vector.tensor_scalar_add(nm, nm, PSCALE)
        p_sb = p_p.tile([128, S], F16, tag="p")
        ssum4 = p_stat.tile([128, SB], F32, tag="ssum4")
        for sb in range(SB):
            nc.scalar.activation(p_sb[:, sb * 512:(sb + 1) * 512], score_ps[sb],
                                 AF.Exp, bias=nm, scale=1.0,
                                 accum_out=ssum4[:, sb:sb + 1])
        ssum = p_stat.tile([128, 1], F32, tag="ssum")
        nc.vector.reduce_sum(ssum, ssum4, axis=AX.X)
        rinv = p_stat.tile([128, 1], F32, tag="rinv")
        nc.vector.reciprocal(rinv, ssum)
        pt = p_pt.tile([128, ST, 128], F16, tag="pt")
        nc.sync.dma_start(pt, p_sb, transpose=True)
        return pt, rinv

    def emit_pv(lt, pt, rinv):
        out_ps = [ps_out.tile([128, 512], F32, tag="o", name=f"ops{lt}_{i}")
                  for i in range(DB)]
        for sc in range(ST):
            for dc in range(DB):
                nc.tensor.matmul(out_ps[dc], pt[:, sc, :],
                                 v_sb[sc // 4][:, sc % 4, dc * 512:(dc + 1) * 512],
                                 start=(sc == 0), stop=(sc == ST - 1))
        o_sb = p_out.tile([128, D], F32, tag="osb")
        for dc in range(DB):
            nc.vector.tensor_scalar_mul(o_sb[:, dc * 512:(dc + 1) * 512],
                                        out_ps[dc], rinv)
        nc.scalar.dma_start(out[lt * 128:(lt + 1) * 128, :], o_sb)

    pending = None
    for lt in range(LT):
        cur = emit_score_softmax(lt)
        if ablate is None:
            prefetch(lt)
        if pending is not None:
            emit_pv(lt - 1, *pending)
        pending = cur
    emit_pv(LT - 1, *pending)


_CACHE = {}


def _build(reps=1, loop_T=0, loop_all=0, only=None):
    key_ = (reps, loop_T, loop_all, only)
    if key_ in _CACHE:
        return _CACHE[key_]
    nc = bacc.Bacc("TRN2", target_bir_lowering=False, debug=False,
                   num_devices=N_CORES)
    query = nc.dram_tensor("query", [L, D], F32, kind="ExternalInput").ap()
    key = nc.dram_tensor("key", [S, D], F32, kind="ExternalInput").ap()
    value = nc.dram_tensor("value", [S, D], F32, kind="ExternalInput").ap()
    W = nc.dram_tensor("W", [D, D], F32, kind="ExternalInput").ap()
    b = nc.dram_tensor("b", [D], F32, kind="ExternalInput").ap()
    out = nc.dram_tensor("out", [L, D], F32, kind="ExternalOutput").ap()
    tag = None
    loop_T = loop_T or loop_all
    if reps > 1 or loop_T:
        # distinct I/O signature per variant so the neuron compile cache
        # (keyed on HLO structure, not backend_config) can't collide
        tag = nc.dram_tensor("tag", [8, reps * 100 + max(loop_T, 1)], F32,
                             kind="ExternalOutput").ap()
    with tile.TileContext(nc) as tc:
        if loop_all and only in (None, "v5b", "v5c"):
            # steady-state software-pipelined loop (see v5 notes above)
            with ExitStack() as ctx:
                st = _v5_state(ctx, tc)
                _v5_prologue(tc, st, query, key, W, b)
                with tc.For_i(0, loop_all, 1):
                    with ExitStack() as bctx:
                        _v5_body(bctx, tc, st, query, key, value, W, b, out,
                                 ablate=only)
        elif loop_all:
            with tc.For_i(0, loop_all, 1):
                with ExitStack() as ctx:
                    _emit(ctx, tc, query, key, value, W, b, out, only=only)
        else:
            for _ in range(reps):
                with ExitStack() as ctx:
                    _emit(ctx, tc, query, key, value, W, b, out, loop_T=loop_T,
                          only=only)
        if tag is not None:
            with tc.tile_pool(name="tagp", bufs=1) as tp:
                t = tp.tile([8, reps * 100 + max(loop_T, 1)], F32)
                nc.vector.memset(t, 1.0)
                nc.sync.dma_start(tag, t)
    nc.compile()
    _CACHE[key_] = nc
    return nc


def kernel(key, query, value, W, b):
    key = np.ascontiguousarray(np.asarray(key), dtype=np.float32)
    query = np.ascontiguousarray(np.asarray(query), dtype=np.float32)
    value = np.ascontiguousarray(np.asarray(value), dtype=np.float32)
    W = np.ascontiguousarray(np.asarray(W), dtype=np.float32)
    b = np.ascontiguousarray(np.asarray(b), dtype=np.float32)
    nc = _build()
    in_maps = [
        {"query": query[i], "key": key[i], "value": value[i], "W": W, "b": b}
        for i in range(N_CORES)
    ]
    res = bass_utils.run_bass_kernel_spmd(nc, in_maps, core_ids=list(range(N_CORES)))
    return np.stack([res.results[i]["out"] for i in range(N_CORES)], axis=0)
